# revision 1
# baseline (speedup 1.0000x reference)
"""Ball attention (block-local attention, ball size 128) on 8 Trainium2 cores.

Reference computation (per (b,h) head, per ball of 128 consecutive tokens):
    S = Q K^T / sqrt(64);  P = softmax(S, axis=-1);  O = P V

Sharding: the 64 (b,h) heads are split 8-per-core (pure data parallel).

Per-core design (all sizes measured on HW):
  * Loads/stores use the natural ball layout (seq position on partitions).
  * Q^T/K^T via packed 2-ball PE transposes: input [128 seq, 2ball x 64d]
    -> output [2ball x 64d partitions, 128 seq]; ball b of a pair lands on
    partition half 64b. ~173ns per transpose op (PE SBUF latency bound).
  * PSUM->SBUF copies round to float32r (DVE for Q^T, ACT for K^T).
  * S^T = K Q^T as float32r "junk-pair" matmuls: rhs = [qt(ball) | qt(ball+2)]
    gives N=256 which runs at 1 cyc/row (vs 4 for fp32); half the output is
    discarded. Measured 274ns/ball, rel err ~1.7e-4 on S (float32r rounds
    mantissas; final output error ~3e-5).
  * E = exp(S^T/8) on ACT directly into SBUF; the ones-column appended to V
    provides softmax denominators through the O matmul; normalize on DVE via
    a per-partition reciprocal broadcast.
  * O matmul dtype selectable (BALL_O_DTYPE): bf16 (fast, adds ~1e-3 error),
    float32r, or float32.
"""

import os
import sys

for _p in ("/opt/trn_rl_repo",):
    if _p not in sys.path and os.path.isdir(_p):
        sys.path.insert(0, _p)

from contextlib import ExitStack

import numpy as np

import concourse.bass as bass
import concourse.mybir as mybir
import concourse.tile as tile
from concourse import bacc
from concourse._compat import with_exitstack
from concourse.masks import make_identity

B, H, N, DH = 4, 16, 8192, 64
BS = 128                 # ball size == SBUF partition count
NCORES = 8
HEADS = B * H // NCORES  # heads per core (8)
M = N // BS              # balls per head (64)

FP32 = mybir.dt.float32
FP32R = mybir.dt.float32r
BF16 = mybir.dt.bfloat16

S_MODE = os.environ.get("BALL_S_MODE", "fp32r")   # fp32r (fast, rel err 2e-4) | fp32 (rel err 1e-5)
TQK = "pe"  # DVE StreamTranspose path abandoned (AP adjacency limits)
O_MODE = os.environ.get("BALL_O_MODE", "fp32")    # bf16 | fp32r | fp32
GRP = 4
# debug bisect: 1=transposes+copies, 2=+S+exp, 3=+O, 4=full (default)
STAGE = int(os.environ.get("BALL_STAGE", "4"))


@with_exitstack
def ball_attention_kernel(
    ctx: ExitStack,
    tc: tile.TileContext,
    out_ap: bass.AP,
    q_ap: bass.AP,
    k_ap: bass.AP,
    v_ap: bass.AP,
    heads: int = HEADS,
    m: int = M,
):
    nc = tc.nc
    assert m % GRP == 0
    ngrp = m // GRP
    scale = 1.0 / float(np.sqrt(DH))
    t_dt = FP32R if S_MODE == "fp32r" else FP32
    if O_MODE == "bf16":
        e_dt = v_dt = BF16
    elif O_MODE == "fp32r":
        e_dt = v_dt = FP32R
    else:
        e_dt = v_dt = FP32

    const_pool = ctx.enter_context(tc.tile_pool(name="const", bufs=1))
    io_pool = ctx.enter_context(tc.tile_pool(name="io", bufs=2))
    t_sb_pool = ctx.enter_context(tc.tile_pool(name="t_sb", bufs=3))
    e_pool = ctx.enter_context(tc.tile_pool(name="e", bufs=2))
    r_pool = ctx.enter_context(tc.tile_pool(name="r", bufs=2))
    t_ps_pool = ctx.enter_context(tc.tile_pool(name="t_ps", bufs=2, space="PSUM"))
    s_ps_pool = ctx.enter_context(tc.tile_pool(name="s_ps", bufs=2, space="PSUM"))
    o_ps_pool = ctx.enter_context(tc.tile_pool(name="o_ps", bufs=2, space="PSUM"))

    ident = const_pool.tile([BS, BS], FP32)
    make_identity(nc, ident)

    nchunk = int(os.environ.get("BALL_NCHUNK", "2"))  # head-load split
    mc = m // nchunk
    for h in range(heads):
        # ---- loads (natural ball layout: partition = seq within ball) -----
        # spread across the three DMA paths: Q on the SP HWDGE ring, K on the
        # ACT HWDGE ring, V (+ the output store) on SWDGE.
        if TQK == "dve":
            q_sb = io_pool.tile([BS, m // 2, 4, 32], FP32, tag="q")
            k_sb = io_pool.tile([BS, m // 2, 4, 32], FP32, tag="k")
        else:
            q_sb = io_pool.tile([BS, m, DH], FP32, tag="q")
            k_sb = io_pool.tile([BS, m, DH], FP32, tag="k")
        vt = io_pool.tile([BS, m, DH + 1], v_dt, tag="vt")
        if TQK == "dve":
            # staged layout for DVE 32x32 block transposes: partition
            # (64*par + 32*db + al) <- row 32*sb+al, d 32*db+be of ball 2a+par
            qv_ = q_ap[h].rearrange(
                "(a par sb al) (db be) -> (par db al) a sb be",
                par=2, sb=4, al=32, db=2,
            )
            kv_ = k_ap[h].rearrange(
                "(a par sb al) (db be) -> (par db al) a sb be",
                par=2, sb=4, al=32, db=2,
            )
        else:
            qv_ = q_ap[h].rearrange("(mm p) d -> p mm d", p=BS)
            kv_ = k_ap[h].rearrange("(mm p) d -> p mm d", p=BS)
        vv_ = v_ap[h].rearrange("(mm p) d -> p mm d", p=BS)
        for c in range(nchunk):
            cs = slice(c * mc, (c + 1) * mc)
            if TQK == "dve":
                cs2 = slice(c * mc // 2, (c + 1) * mc // 2)
                nc.sync.dma_start(q_sb[:, cs2, :, :], qv_[:, cs2, :, :])
                nc.sync.dma_start(k_sb[:, cs2, :, :], kv_[:, cs2, :, :])
            else:
                nc.sync.dma_start(q_sb[:, cs, :], qv_[:, cs, :])
                nc.sync.dma_start(k_sb[:, cs, :], kv_[:, cs, :])
            nc.sync.dma_start(vt[:, cs, 0:DH], vv_[:, cs, :])
        nc.vector.memset(vt[:, :, DH], 1.0)
        ob = io_pool.tile([BS, m, DH], FP32, tag="ob")

        for g in range(ngrp):
            # 4 balls: m0..m0+3; junk-pairs (m0, m0+2) and (m0+1, m0+3)
            m0 = g * GRP
            qt = t_sb_pool.tile([BS, 2, BS], t_dt, tag="qt")        # [pair, seq]
            kt = t_sb_pool.tile([BS, 2, BS], t_dt, tag="kt")
            if TQK == "dve":
                # DVE 32x32 block transposes of the staged tiles give the
                # packed [64*par + d, seq] layout directly, SBUF->SBUF.
                a0 = m0 // 2
                nc.vector.transpose(qt[:, 0, :], q_sb[:, a0, :, :])
                nc.vector.transpose(kt[:, 0, :], k_sb[:, a0, :, :])
                nc.vector.transpose(qt[:, 1, :], q_sb[:, a0 + 1, :, :])
                nc.vector.transpose(kt[:, 1, :], k_sb[:, a0 + 1, :, :])
            else:
                # one bank: [qt(pair0) | kt(pair0) | qt(pair1) | kt(pair1)]
                t_ps = t_ps_pool.tile([BS, 4, BS], FP32, tag="t")
                # packed transposes: 2 balls/op; ball parity b -> partitions 64b
                nc.tensor.transpose(t_ps[:, 0, :], q_sb[:, m0 : m0 + 2, :], ident)
                nc.tensor.transpose(t_ps[:, 1, :], k_sb[:, m0 : m0 + 2, :], ident)
                nc.tensor.transpose(t_ps[:, 2, :], q_sb[:, m0 + 2 : m0 + 4, :], ident)
                nc.tensor.transpose(t_ps[:, 3, :], k_sb[:, m0 + 2 : m0 + 4, :], ident)
                # PSUM -> SBUF (+ round to fp32r): DVE takes Q^T, ACT takes K^T
                nc.vector.tensor_copy(qt, t_ps[:, 0:4:2, :])
                nc.scalar.copy(kt, t_ps[:, 1:4:2, :])
            if STAGE == 1:
                nc.vector.tensor_copy(
                    ob[:, m0 : m0 + 2, :], qt[:, :, 0:DH].bitcast(FP32)
                )
                nc.vector.tensor_copy(
                    ob[:, m0 + 2 : m0 + 4, :], kt[:, :, 0:DH].bitcast(FP32)
                )
                continue

            # S^T matmuls. qt slot layout: [pair a' = 0|1][seq], ball (2j+b)
            # at partitions 64b. junk-pair rhs = qt[64b:64b+64, :, :] (N=256).
            # Consecutive matmuls must hit different PSUM banks (same-bank
            # back-to-back PE matmul writes fault): ball j -> bank j%2,
            # slot j//2 of a 2-bank tile.
            s_ps = s_ps_pool.tile([BS, 2, 2, 2 * BS], FP32, tag="s", bufs=int(os.environ.get("BALL_SBUFS", "2")))
            e_sb = e_pool.tile([BS, GRP, BS], e_dt, tag="e")
            for j in range(GRP):
                a2, b = j >> 1, j & 1          # ball m0+j = pair a2, parity b
                lo = 64 * b
                if S_MODE == "fp32r":
                    nc.tensor.matmul(
                        s_ps[:, j % 2, j // 2, :],
                        kt[lo : lo + 64, a2, :],
                        qt[lo : lo + 64, :, :],
                        start=True,
                        stop=True,
                    )
                else:
                    nc.tensor.matmul(
                        s_ps[:, j % 2, j // 2, a2 * BS : a2 * BS + BS],
                        kt[lo : lo + 64, a2, :],
                        qt[lo : lo + 64, a2, :],
                        start=True,
                        stop=True,
                    )
            if STAGE == 15:
                for a2 in range(2):
                    nc.vector.tensor_copy(
                        ob[:, m0 + a2 * 2 : m0 + a2 * 2 + 2, :],
                        s_ps[:, :, a2, a2 * BS : a2 * BS + DH],
                    )
                continue
            # E = exp(S^T/8); good half of ball j's junk-pair output is the
            # column block of its own pair slot (a2 = j>>1 = tile slot dim).
            for a2 in range(2):
                nc.scalar.activation(
                    e_sb[:, a2 * 2 : a2 * 2 + 2, :],
                    s_ps[:, :, a2, a2 * BS : a2 * BS + BS],
                    mybir.ActivationFunctionType.Exp,
                    scale=scale,
                )

            if STAGE == 2:
                if O_MODE == "bf16":
                    for j in range(GRP):
                        nc.vector.tensor_copy(ob[:, m0 + j, :], e_sb[:, j, 0:DH])
                else:
                    nc.vector.tensor_copy(ob[:, m0 : m0 + GRP, :], e_sb[:, :, 0:DH].bitcast(FP32))
                continue

            # O_unnorm = E^T @ [V | 1]
            o_ps = o_ps_pool.tile([BS, GRP, DH + 1], FP32, tag="o")
            for j in range(GRP):
                nc.tensor.matmul(
                    o_ps[:, j, :],
                    e_sb[:, j, :],
                    vt[:, m0 + j, :],
                    start=True,
                    stop=True,
                )
            if STAGE == 3:
                nc.vector.tensor_copy(ob[:, m0 : m0 + GRP, :], o_ps[:, :, 0:DH])
                continue
            # normalize by the ones-column sums
            r_sb = r_pool.tile([BS, GRP], FP32, tag="r")
            nc.vector.reciprocal(r_sb, o_ps[:, :, DH])
            nc.vector.tensor_mul(
                ob[:, m0 : m0 + GRP, :],
                o_ps[:, :, 0:DH],
                r_sb.unsqueeze(2).broadcast_to([BS, GRP, DH]),
            )

        # ---- store -------------------------------------------------------
        nc.gpsimd.dma_start(out_ap[h].rearrange("(mm p) d -> p mm d", p=BS), ob)


def build_nc(heads: int = HEADS, m: int = M):
    nc = bacc.Bacc("TRN2", target_bir_lowering=False, debug=False, num_devices=NCORES)
    q = nc.dram_tensor("q", [heads, m * BS, DH], FP32, kind="ExternalInput").ap()
    k = nc.dram_tensor("k", [heads, m * BS, DH], FP32, kind="ExternalInput").ap()
    v = nc.dram_tensor("v", [heads, m * BS, DH], FP32, kind="ExternalInput").ap()
    o = nc.dram_tensor("out", [heads, m * BS, DH], FP32, kind="ExternalOutput").ap()
    with tile.TileContext(nc) as tc:
        ball_attention_kernel(tc, o, q, k, v, heads=heads, m=m)
    nc.compile()
    return nc


_NC_CACHE = {}


def kernel(q: np.ndarray, k: np.ndarray, v: np.ndarray) -> np.ndarray:
    from concourse.bass_utils import run_bass_kernel_spmd

    assert q.shape == (B, H, N, DH)
    if "nc" not in _NC_CACHE:
        _NC_CACHE["nc"] = build_nc()
    nc = _NC_CACHE["nc"]

    hpc = HEADS
    qf = np.ascontiguousarray(np.asarray(q, dtype=np.float32).reshape(B * H, N, DH))
    kf = np.ascontiguousarray(np.asarray(k, dtype=np.float32).reshape(B * H, N, DH))
    vf = np.ascontiguousarray(np.asarray(v, dtype=np.float32).reshape(B * H, N, DH))
    in_maps = [
        {
            "q": np.ascontiguousarray(qf[c * hpc : (c + 1) * hpc]),
            "k": np.ascontiguousarray(kf[c * hpc : (c + 1) * hpc]),
            "v": np.ascontiguousarray(vf[c * hpc : (c + 1) * hpc]),
        }
        for c in range(NCORES)
    ]
    res = run_bass_kernel_spmd(nc, in_maps, core_ids=list(range(NCORES)))
    out = np.concatenate([res.results[c]["out"] for c in range(NCORES)], axis=0)
    return out.reshape(B, H, N, DH)



# revision 3
# speedup vs baseline: 3.1817x; 3.1817x over previous
"""Ball attention (block-local attention, ball size 128) on 8 Trainium2 cores.

Reference computation (per (b,h) head, per ball of 128 consecutive tokens):
    S = Q K^T / sqrt(64);  P = softmax(S, axis=-1);  O = P V

Sharding: the 64 (b,h) heads are split 8-per-core (pure data parallel).

v2 design — memory-roofline oriented:
  * All device I/O in fp16 (HBM traffic 33.7 MB/core vs 67 MB in fp32;
    fp16 keeps ~5e-4 output error, far under the 2e-2 gate).
  * Q and K are transposed on the HOST into the packed-pair layout
    [head, 64*(ball%2)+d, ball//2, seq] so the kernel needs NO PE
    transposes and no PSUM->SBUF transpose copies (v1's PE was 84% busy
    and HAM-throttled 75% of the time largely due to transpose-mode ops).
  * V is repacked host-side to [head, seq, ball, 65] with a ones column
    baked in at d=64: softmax denominators fall out of the O matmul.
  * S^T per ball via one K=64 matmul; even/odd balls sit in PE row
    groups (0,0)/(64,0) (auto-derived from base_partition) and execute
    concurrently in the array.
  * exp(S/8) on ACT in one instruction per 8 balls (N=1024) to amortize
    the ~293ns fixed ACT instruction cost.
  * O = E^T @ [V|1] per ball; normalize on DVE via reciprocal+broadcast
    multiply; store fp16, host upcasts.
"""

import os
import sys

for _p in ("/opt/trn_rl_repo",):
    if _p not in sys.path and os.path.isdir(_p):
        sys.path.insert(0, _p)

from contextlib import ExitStack

import numpy as np

import concourse.bass as bass
import concourse.mybir as mybir
import concourse.tile as tile
from concourse import bacc
from concourse._compat import with_exitstack

B, H, N, DH = 4, 16, 8192, 64
BS = 128                 # ball size == SBUF partition count
NCORES = 8
HEADS = B * H // NCORES  # heads per core (8)
M = N // BS              # balls per head (64)
PAIRS = M // 2           # packed ball pairs (32)

FP32 = mybir.dt.float32
FP16 = mybir.dt.float16

SETB = 8                 # balls per pipeline set (one ACT exp instr each)
NSETS = M // SETB
IO_BUFS = int(os.environ.get("BALL_IO_BUFS", "2"))


@with_exitstack
def ball_attention_kernel(
    ctx: ExitStack,
    tc: tile.TileContext,
    out_ap: bass.AP,
    q_ap: bass.AP,
    k_ap: bass.AP,
    v_ap: bass.AP,
    heads: int = HEADS,
    m: int = M,
):
    nc = tc.nc
    scale = 1.0 / float(np.sqrt(DH))

    io_pool = ctx.enter_context(tc.tile_pool(name="io", bufs=IO_BUFS))
    e_pool = ctx.enter_context(tc.tile_pool(name="e", bufs=2))
    r_pool = ctx.enter_context(tc.tile_pool(name="r", bufs=2))
    s_ps_pool = ctx.enter_context(tc.tile_pool(name="s_ps", bufs=2, space="PSUM"))
    o_ps_pool = ctx.enter_context(tc.tile_pool(name="o_ps", bufs=2, space="PSUM"))

    nsets = m // SETB

    for h in range(heads):
        # ---- loads: per-partition lines are fully contiguous in HBM ----
        qt = io_pool.tile([BS, PAIRS, BS], FP16, tag="qt")   # [64b+d, pair, seq]
        kt = io_pool.tile([BS, PAIRS, BS], FP16, tag="kt")
        vt = io_pool.tile([BS, m, DH + 1], FP16, tag="vt")   # [seq, ball, d|1]
        ob = io_pool.tile([BS, m, DH], FP16, tag="ob")
        nc.sync.dma_start(qt, q_ap[h])
        nc.sync.dma_start(kt, k_ap[h])
        nc.sync.dma_start(vt, v_ap[h])

        def do_o(s0, e_sb, ob=ob, vt=vt):
            # O_unnorm = E^T @ [V|1]; ball j -> PSUM bank j%2 slot j//2.
            # Slot stride padded to 128 floats so every matmul output stays
            # inside one 2KB bank (65-float slots would cross at slot 3).
            o_ps = o_ps_pool.tile([BS, 2, SETB // 2, BS], FP32, tag="o")
            for j in range(SETB):
                nc.tensor.matmul(
                    o_ps[:, j % 2, j // 2, 0 : DH + 1],
                    e_sb[:, j % 2, j // 2, :],
                    vt[:, s0 + j, :],
                    start=True,
                    stop=True,
                )
            # normalize by the ones-column sums
            r_sb = r_pool.tile([BS, SETB], FP32, tag="r")
            nc.vector.reciprocal(r_sb, o_ps[:, :, :, DH])
            half = SETB // 2
            for b in range(2):
                nc.vector.tensor_mul(
                    ob[:, s0 + b : s0 + SETB : 2, :],
                    o_ps[:, b, :, 0:DH],
                    r_sb[:, half * b : half * b + half]
                    .unsqueeze(2)
                    .broadcast_to([BS, half, DH]),
                )

        pend = None
        for s in range(nsets):
            s0 = s * SETB
            # S^T matmuls: ball j contracts over its 64 d-partitions
            # (parity b -> partitions 64b, PE row group auto-derived).
            # Consecutive matmuls alternate PSUM banks (bank j%2).
            s_ps = s_ps_pool.tile([BS, 2, SETB // 2, BS], FP32, tag="s")
            for j in range(SETB):
                ball = s0 + j
                pair, par = ball >> 1, ball & 1
                lo = 64 * par
                nc.tensor.matmul(
                    s_ps[:, j % 2, j // 2, :],
                    kt[lo : lo + 64, pair, :],
                    qt[lo : lo + 64, pair, :],
                    start=True,
                    stop=True,
                )
            # E = exp(S^T/8): one ACT op over both banks (N=1024)
            e_sb = e_pool.tile([BS, 2, SETB // 2, BS], FP16, tag="e")
            nc.scalar.activation(
                e_sb, s_ps, mybir.ActivationFunctionType.Exp, scale=scale
            )
            # software pipeline: O for the previous set runs while this
            # set's exp is on ACT
            if pend is not None:
                do_o(*pend)
            pend = (s0, e_sb)
        do_o(*pend)

        # ---- store (SWDGE ring; loads are on the HWDGE rings) ----------
        nc.gpsimd.dma_start(out_ap[h], ob)


def build_nc(heads: int = HEADS, m: int = M):
    nc = bacc.Bacc("TRN2", target_bir_lowering=False, debug=False, num_devices=NCORES)
    q = nc.dram_tensor("q", [heads, BS, PAIRS, BS], FP16, kind="ExternalInput").ap()
    k = nc.dram_tensor("k", [heads, BS, PAIRS, BS], FP16, kind="ExternalInput").ap()
    v = nc.dram_tensor("v", [heads, BS, m, DH + 1], FP16, kind="ExternalInput").ap()
    o = nc.dram_tensor("out", [heads, BS, m, DH], FP16, kind="ExternalOutput").ap()
    with tile.TileContext(nc) as tc:
        ball_attention_kernel(tc, o, q, k, v, heads=heads, m=m)
    nc.compile()
    return nc


_NC_CACHE = {}


def _pack_qk(x: np.ndarray) -> np.ndarray:
    """[64, N, DH] fp32 -> [64, 128(=64*par+d), 32 pair, 128 seq] fp16."""
    xh = x.astype(np.float16)
    xh = xh.reshape(B * H, PAIRS, 2, BS, DH)          # h, pair, par, s, d
    xh = xh.transpose(0, 2, 4, 1, 3)                   # h, par, d, pair, s
    return np.ascontiguousarray(xh.reshape(B * H, BS, PAIRS, BS))


def _pack_v(x: np.ndarray) -> np.ndarray:
    """[64, N, DH] fp32 -> [64, 128 seq, 64 ball, 65] fp16 with ones col."""
    xh = x.astype(np.float16)
    xh = xh.reshape(B * H, M, BS, DH).transpose(0, 2, 1, 3)  # h, s, ball, d
    out = np.empty((B * H, BS, M, DH + 1), dtype=np.float16)
    out[..., :DH] = xh
    out[..., DH] = np.float16(1.0)
    return out


def kernel(q: np.ndarray, k: np.ndarray, v: np.ndarray) -> np.ndarray:
    from concourse.bass_utils import run_bass_kernel_spmd

    assert q.shape == (B, H, N, DH)
    if "nc" not in _NC_CACHE:
        _NC_CACHE["nc"] = build_nc()
    nc = _NC_CACHE["nc"]

    qt = _pack_qk(np.asarray(q, dtype=np.float32).reshape(B * H, N, DH))
    kt = _pack_qk(np.asarray(k, dtype=np.float32).reshape(B * H, N, DH))
    vt = _pack_v(np.asarray(v, dtype=np.float32).reshape(B * H, N, DH))
    hpc = HEADS
    in_maps = [
        {
            "q": np.ascontiguousarray(qt[c * hpc : (c + 1) * hpc]),
            "k": np.ascontiguousarray(kt[c * hpc : (c + 1) * hpc]),
            "v": np.ascontiguousarray(vt[c * hpc : (c + 1) * hpc]),
        }
        for c in range(NCORES)
    ]
    res = run_bass_kernel_spmd(nc, in_maps, core_ids=list(range(NCORES)))
    out = np.concatenate([res.results[c]["out"] for c in range(NCORES)], axis=0)
    # [64, seq, ball, d] fp16 -> [B, H, N, DH] fp32
    out = out.transpose(0, 2, 1, 3).reshape(B, H, N, DH)
    return out.astype(np.float32)


# revision 7
# speedup vs baseline: 3.4954x; 1.0986x over previous
"""Ball attention (block-local attention, ball size 128) on 8 Trainium2 cores.

Reference computation (per (b,h) head, per ball of 128 consecutive tokens):
    S = Q K^T / sqrt(64);  P = softmax(S, axis=-1);  O = P V

Sharding: the 64 (b,h) heads are split 8-per-core (pure data parallel).

v2 design — memory-roofline oriented:
  * All device I/O in fp16 (HBM traffic 33.7 MB/core vs 67 MB in fp32;
    fp16 keeps ~5e-4 output error, far under the 2e-2 gate).
  * Q and K are transposed on the HOST into the packed-pair layout
    [head, 64*(ball%2)+d, ball//2, seq] so the kernel needs NO PE
    transposes and no PSUM->SBUF transpose copies (v1's PE was 84% busy
    and HAM-throttled 75% of the time largely due to transpose-mode ops).
  * V is repacked host-side to [head, seq, ball, 65] with a ones column
    baked in at d=64: softmax denominators fall out of the O matmul.
  * S^T per ball via one K=64 matmul; even/odd balls sit in PE row
    groups (0,0)/(64,0) (auto-derived from base_partition) and execute
    concurrently in the array.
  * exp(S/8) on ACT in one instruction per 8 balls (N=1024) to amortize
    the ~293ns fixed ACT instruction cost.
  * O = E^T @ [V|1] per ball; normalize on DVE via reciprocal+broadcast
    multiply; store fp16, host upcasts.
"""

import os
import sys

for _p in ("/opt/trn_rl_repo",):
    if _p not in sys.path and os.path.isdir(_p):
        sys.path.insert(0, _p)

from contextlib import ExitStack

import numpy as np

import concourse.bass as bass
import concourse.mybir as mybir
import concourse.tile as tile
from concourse import bacc
from concourse._compat import with_exitstack

B, H, N, DH = 4, 16, 8192, 64
BS = 128                 # ball size == SBUF partition count
NCORES = 8
HEADS = B * H // NCORES  # heads per core (8)
M = N // BS              # balls per head (64)
PAIRS = M // 2           # packed ball pairs (32)

FP32 = mybir.dt.float32
FP16 = mybir.dt.float16

SETB = 8                 # balls per pipeline set (one ACT exp instr each)
NSETS = M // SETB
IO_BUFS = int(os.environ.get("BALL_IO_BUFS", "3"))
NCHUNK = int(os.environ.get("BALL_NCHUNK", "4"))      # load chunks per head
STORE_SETS = int(os.environ.get("BALL_STORE_SETS", "2"))  # sets per store DMA


@with_exitstack
def ball_attention_kernel(
    ctx: ExitStack,
    tc: tile.TileContext,
    out_ap: bass.AP,
    q_ap: bass.AP,
    k_ap: bass.AP,
    v_ap: bass.AP,
    heads: int = HEADS,
    m: int = M,
):
    nc = tc.nc
    scale = 1.0 / float(np.sqrt(DH))

    io_pool = ctx.enter_context(tc.tile_pool(name="io", bufs=IO_BUFS))
    e_pool = ctx.enter_context(tc.tile_pool(name="e", bufs=2))
    r_pool = ctx.enter_context(tc.tile_pool(name="r", bufs=2))
    s_ps_pool = ctx.enter_context(tc.tile_pool(name="s_ps", bufs=2, space="PSUM"))
    o_ps_pool = ctx.enter_context(tc.tile_pool(name="o_ps", bufs=2, space="PSUM"))

    nsets = m // SETB

    for h in range(heads):
        # ---- loads: per-partition lines are fully contiguous in HBM ----
        qt = io_pool.tile([BS, PAIRS, BS], FP16, tag="qt")   # [64b+d, pair, seq]
        kt = io_pool.tile([BS, PAIRS, BS], FP16, tag="kt")
        vt = io_pool.tile([BS, m, DH + 1], FP16, tag="vt")   # [seq, ball, d|1]
        ob = io_pool.tile([BS, m, DH], FP16, tag="ob")
        # chunked loads: compute on the first pairs can start before the
        # rest of the head arrives (Tile tracks subregion deps)
        pc, mc = PAIRS // NCHUNK, m // NCHUNK
        for c in range(NCHUNK):
            ps, ms = slice(c * pc, (c + 1) * pc), slice(c * mc, (c + 1) * mc)
            nc.sync.dma_start(qt[:, ps, :], q_ap[h][:, ps, :])
            nc.sync.dma_start(kt[:, ps, :], k_ap[h][:, ps, :])
            nc.sync.dma_start(vt[:, ms, :], v_ap[h][:, ms, :])

        def do_o(s0, e_sb, ob=ob, vt=vt):
            # O_unnorm = E^T @ [V|1]; ball j -> PSUM bank j%2 slot j//2.
            # Slot stride padded to 128 floats so every matmul output stays
            # inside one 2KB bank (65-float slots would cross at slot 3).
            o_ps = o_ps_pool.tile([BS, 2, SETB // 2, BS], FP32, tag="o")
            for j in range(SETB):
                nc.tensor.matmul(
                    o_ps[:, j % 2, j // 2, 0 : DH + 1],
                    e_sb[:, j % 2, j // 2, :],
                    vt[:, s0 + j, :],
                    start=True,
                    stop=True,
                )
            # normalize by the ones-column sums
            r_sb = r_pool.tile([BS, SETB], FP32, tag="r")
            nc.vector.reciprocal(r_sb, o_ps[:, :, :, DH])
            half = SETB // 2
            for b in range(2):
                nc.vector.tensor_mul(
                    ob[:, s0 + b : s0 + SETB : 2, :],
                    o_ps[:, b, :, 0:DH],
                    r_sb[:, half * b : half * b + half]
                    .unsqueeze(2)
                    .broadcast_to([BS, half, DH]),
                )

        def store_upto(t):
            # stream out finished sets so the final store tail is short
            if (t + 1) % STORE_SETS == 0:
                lo = (t + 1 - STORE_SETS) * SETB
                hi = (t + 1) * SETB
                nc.gpsimd.dma_start(out_ap[h][:, lo:hi, :], ob[:, lo:hi, :])

        pend = None
        for s in range(nsets):
            s0 = s * SETB
            # S^T matmuls: ball j contracts over its 64 d-partitions
            # (parity b -> partitions 64b, PE row group auto-derived).
            # Consecutive matmuls alternate PSUM banks (bank j%2).
            s_ps = s_ps_pool.tile([BS, 2, SETB // 2, BS], FP32, tag="s")
            for j in range(SETB):
                ball = s0 + j
                pair, par = ball >> 1, ball & 1
                lo = 64 * par
                nc.tensor.matmul(
                    s_ps[:, j % 2, j // 2, :],
                    kt[lo : lo + 64, pair, :],
                    qt[lo : lo + 64, pair, :],
                    start=True,
                    stop=True,
                )
            # E = exp(S^T/8): one ACT op over both banks (N=1024)
            e_sb = e_pool.tile([BS, 2, SETB // 2, BS], FP16, tag="e")
            nc.scalar.activation(
                e_sb, s_ps, mybir.ActivationFunctionType.Exp, scale=scale
            )
            # software pipeline: O for the previous set runs while this
            # set's exp is on ACT
            if pend is not None:
                do_o(*pend)
                store_upto(s - 1)
            pend = (s0, e_sb)
        do_o(*pend)
        store_upto(nsets - 1)


def build_nc(heads: int = HEADS, m: int = M):
    nc = bacc.Bacc("TRN2", target_bir_lowering=False, debug=False, num_devices=NCORES)
    q = nc.dram_tensor("q", [heads, BS, PAIRS, BS], FP16, kind="ExternalInput").ap()
    k = nc.dram_tensor("k", [heads, BS, PAIRS, BS], FP16, kind="ExternalInput").ap()
    v = nc.dram_tensor("v", [heads, BS, m, DH + 1], FP16, kind="ExternalInput").ap()
    o = nc.dram_tensor("out", [heads, BS, m, DH], FP16, kind="ExternalOutput").ap()
    with tile.TileContext(nc) as tc:
        ball_attention_kernel(tc, o, q, k, v, heads=heads, m=m)
    nc.compile()
    return nc


_NC_CACHE = {}


def _pack_qk(x: np.ndarray) -> np.ndarray:
    """[64, N, DH] fp32 -> [64, 128(=64*par+d), 32 pair, 128 seq] fp16."""
    xh = x.astype(np.float16)
    xh = xh.reshape(B * H, PAIRS, 2, BS, DH)          # h, pair, par, s, d
    xh = xh.transpose(0, 2, 4, 1, 3)                   # h, par, d, pair, s
    return np.ascontiguousarray(xh.reshape(B * H, BS, PAIRS, BS))


def _pack_v(x: np.ndarray) -> np.ndarray:
    """[64, N, DH] fp32 -> [64, 128 seq, 64 ball, 65] fp16 with ones col."""
    xh = x.astype(np.float16)
    xh = xh.reshape(B * H, M, BS, DH).transpose(0, 2, 1, 3)  # h, s, ball, d
    out = np.empty((B * H, BS, M, DH + 1), dtype=np.float16)
    out[..., :DH] = xh
    out[..., DH] = np.float16(1.0)
    return out


def kernel(q: np.ndarray, k: np.ndarray, v: np.ndarray) -> np.ndarray:
    from concourse.bass_utils import run_bass_kernel_spmd

    assert q.shape == (B, H, N, DH)
    if "nc" not in _NC_CACHE:
        _NC_CACHE["nc"] = build_nc()
    nc = _NC_CACHE["nc"]

    qt = _pack_qk(np.asarray(q, dtype=np.float32).reshape(B * H, N, DH))
    kt = _pack_qk(np.asarray(k, dtype=np.float32).reshape(B * H, N, DH))
    vt = _pack_v(np.asarray(v, dtype=np.float32).reshape(B * H, N, DH))
    hpc = HEADS
    in_maps = [
        {
            "q": np.ascontiguousarray(qt[c * hpc : (c + 1) * hpc]),
            "k": np.ascontiguousarray(kt[c * hpc : (c + 1) * hpc]),
            "v": np.ascontiguousarray(vt[c * hpc : (c + 1) * hpc]),
        }
        for c in range(NCORES)
    ]
    res = run_bass_kernel_spmd(nc, in_maps, core_ids=list(range(NCORES)))
    out = np.concatenate([res.results[c]["out"] for c in range(NCORES)], axis=0)
    # [64, seq, ball, d] fp16 -> [B, H, N, DH] fp32
    out = out.transpose(0, 2, 1, 3).reshape(B, H, N, DH)
    return out.astype(np.float32)


# revision 9
# speedup vs baseline: 3.8215x; 1.0933x over previous
"""Ball attention (block-local attention, ball size 128) on 8 Trainium2 cores.

Reference computation (per (b,h) head, per ball of 128 consecutive tokens):
    S = Q K^T / sqrt(64);  P = softmax(S, axis=-1);  O = P V

Sharding: the 64 (b,h) heads are split 8-per-core (pure data parallel).

v2 design — memory-roofline oriented:
  * All device I/O in fp16 (HBM traffic 33.7 MB/core vs 67 MB in fp32;
    fp16 keeps ~5e-4 output error, far under the 2e-2 gate).
  * Q and K are transposed on the HOST into the packed-pair layout
    [head, 64*(ball%2)+d, ball//2, seq] so the kernel needs NO PE
    transposes and no PSUM->SBUF transpose copies (v1's PE was 84% busy
    and HAM-throttled 75% of the time largely due to transpose-mode ops).
  * V is repacked host-side to [head, seq, ball, 65] with a ones column
    baked in at d=64: softmax denominators fall out of the O matmul.
  * S^T per ball via one K=64 matmul; even/odd balls sit in PE row
    groups (0,0)/(64,0) (auto-derived from base_partition) and execute
    concurrently in the array.
  * exp(S/8) on ACT in one instruction per 8 balls (N=1024) to amortize
    the ~293ns fixed ACT instruction cost.
  * O = E^T @ [V|1] per ball; normalize on DVE via reciprocal+broadcast
    multiply; store fp16, host upcasts.
"""

import os
import sys

for _p in ("/opt/trn_rl_repo",):
    if _p not in sys.path and os.path.isdir(_p):
        sys.path.insert(0, _p)

from contextlib import ExitStack

import numpy as np

import concourse.bass as bass
import concourse.mybir as mybir
import concourse.tile as tile
from concourse import bacc
from concourse._compat import with_exitstack

B, H, N, DH = 4, 16, 8192, 64
BS = 128                 # ball size == SBUF partition count
NCORES = 8
HEADS = B * H // NCORES  # heads per core (8)
M = N // BS              # balls per head (64)
PAIRS = M // 2           # packed ball pairs (32)

FP32 = mybir.dt.float32
FP16 = mybir.dt.float16

SETB = 8                 # balls per pipeline set (one ACT exp instr each)
NSETS = M // SETB
IO_BUFS = int(os.environ.get("BALL_IO_BUFS", "3"))
NCHUNK = int(os.environ.get("BALL_NCHUNK", "4"))      # load chunks per head
STORE_SETS = int(os.environ.get("BALL_STORE_SETS", "4"))  # sets per store DMA


@with_exitstack
def ball_attention_kernel(
    ctx: ExitStack,
    tc: tile.TileContext,
    out_ap: bass.AP,
    q_ap: bass.AP,
    k_ap: bass.AP,
    v_ap: bass.AP,
    heads: int = HEADS,
    m: int = M,
):
    nc = tc.nc
    scale = 1.0 / float(np.sqrt(DH))

    io_pool = ctx.enter_context(tc.tile_pool(name="io", bufs=IO_BUFS))
    e_pool = ctx.enter_context(tc.tile_pool(name="e", bufs=2))
    r_pool = ctx.enter_context(tc.tile_pool(name="r", bufs=2))
    s_ps_pool = ctx.enter_context(tc.tile_pool(name="s_ps", bufs=2, space="PSUM"))
    o_ps_pool = ctx.enter_context(tc.tile_pool(name="o_ps", bufs=2, space="PSUM"))

    nsets = m // SETB

    for h in range(heads):
        # ---- loads: per-partition lines are fully contiguous in HBM ----
        qt = io_pool.tile([BS, PAIRS, BS], FP16, tag="qt")   # [64b+d, pair, seq]
        kt = io_pool.tile([BS, PAIRS, BS], FP16, tag="kt")
        vt = io_pool.tile([BS, m, DH + 1], FP16, tag="vt")   # [seq, ball, d|1]
        ob = io_pool.tile([BS, m, DH], FP16, tag="ob")
        # chunked loads: compute on the first pairs can start before the
        # rest of the head arrives (Tile tracks subregion deps). Only the
        # pipeline-fill head needs fine chunks; later heads use bigger
        # transfers (better DMA efficiency) since the pipeline hides them.
        nch = NCHUNK if h == 0 else max(NCHUNK // 2, 1)
        pc, mc = PAIRS // nch, m // nch
        for c in range(nch):
            ps, ms = slice(c * pc, (c + 1) * pc), slice(c * mc, (c + 1) * mc)
            nc.sync.dma_start(qt[:, ps, :], q_ap[h][:, ps, :])
            nc.sync.dma_start(kt[:, ps, :], k_ap[h][:, ps, :])
            nc.sync.dma_start(vt[:, ms, :], v_ap[h][:, ms, :])

        def do_o(s0, e_sb, ob=ob, vt=vt):
            # O_unnorm = E^T @ [V|1]; ball j -> PSUM bank j%2 slot j//2.
            # Slot stride padded to 128 floats so every matmul output stays
            # inside one 2KB bank (65-float slots would cross at slot 3).
            o_ps = o_ps_pool.tile([BS, 2, SETB // 2, BS], FP32, tag="o")
            for j in range(SETB):
                nc.tensor.matmul(
                    o_ps[:, j % 2, j // 2, 0 : DH + 1],
                    e_sb[:, j % 2, j // 2, :],
                    vt[:, s0 + j, :],
                    start=True,
                    stop=True,
                )
            # normalize by the ones-column sums
            r_sb = r_pool.tile([BS, SETB], FP32, tag="r")
            nc.vector.reciprocal(r_sb, o_ps[:, :, :, DH])
            half = SETB // 2
            for b in range(2):
                nc.vector.tensor_mul(
                    ob[:, s0 + b : s0 + SETB : 2, :],
                    o_ps[:, b, :, 0:DH],
                    r_sb[:, half * b : half * b + half]
                    .unsqueeze(2)
                    .broadcast_to([BS, half, DH]),
                )

        def store_upto(t):
            # stream out finished sets so the final store tail is short
            if (t + 1) % STORE_SETS == 0:
                lo = (t + 1 - STORE_SETS) * SETB
                hi = (t + 1) * SETB
                nc.gpsimd.dma_start(out_ap[h][:, lo:hi, :], ob[:, lo:hi, :])

        pend = None
        for s in range(nsets):
            s0 = s * SETB
            # S^T matmuls: ball j contracts over its 64 d-partitions
            # (parity b -> partitions 64b, PE row group auto-derived).
            # Consecutive matmuls alternate PSUM banks (bank j%2).
            s_ps = s_ps_pool.tile([BS, 2, SETB // 2, BS], FP32, tag="s")
            for j in range(SETB):
                ball = s0 + j
                pair, par = ball >> 1, ball & 1
                lo = 64 * par
                nc.tensor.matmul(
                    s_ps[:, j % 2, j // 2, :],
                    kt[lo : lo + 64, pair, :],
                    qt[lo : lo + 64, pair, :],
                    start=True,
                    stop=True,
                )
            # E = exp(S^T/8): one ACT op over both banks (N=1024)
            e_sb = e_pool.tile([BS, 2, SETB // 2, BS], FP16, tag="e")
            nc.scalar.activation(
                e_sb, s_ps, mybir.ActivationFunctionType.Exp, scale=scale
            )
            # software pipeline: O for the previous set runs while this
            # set's exp is on ACT
            if pend is not None:
                do_o(*pend)
                store_upto(s - 1)
            pend = (s0, e_sb)
        do_o(*pend)
        store_upto(nsets - 1)


def build_nc(heads: int = HEADS, m: int = M):
    nc = bacc.Bacc("TRN2", target_bir_lowering=False, debug=False, num_devices=NCORES)
    q = nc.dram_tensor("q", [heads, BS, PAIRS, BS], FP16, kind="ExternalInput").ap()
    k = nc.dram_tensor("k", [heads, BS, PAIRS, BS], FP16, kind="ExternalInput").ap()
    v = nc.dram_tensor("v", [heads, BS, m, DH + 1], FP16, kind="ExternalInput").ap()
    o = nc.dram_tensor("out", [heads, BS, m, DH], FP16, kind="ExternalOutput").ap()
    with tile.TileContext(nc) as tc:
        ball_attention_kernel(tc, o, q, k, v, heads=heads, m=m)
    nc.compile()
    return nc


_NC_CACHE = {}


def _pack_qk(x: np.ndarray) -> np.ndarray:
    """[64, N, DH] fp32 -> [64, 128(=64*par+d), 32 pair, 128 seq] fp16."""
    xh = x.astype(np.float16)
    xh = xh.reshape(B * H, PAIRS, 2, BS, DH)          # h, pair, par, s, d
    xh = xh.transpose(0, 2, 4, 1, 3)                   # h, par, d, pair, s
    return np.ascontiguousarray(xh.reshape(B * H, BS, PAIRS, BS))


def _pack_v(x: np.ndarray) -> np.ndarray:
    """[64, N, DH] fp32 -> [64, 128 seq, 64 ball, 65] fp16 with ones col."""
    xh = x.astype(np.float16)
    xh = xh.reshape(B * H, M, BS, DH).transpose(0, 2, 1, 3)  # h, s, ball, d
    out = np.empty((B * H, BS, M, DH + 1), dtype=np.float16)
    out[..., :DH] = xh
    out[..., DH] = np.float16(1.0)
    return out


def kernel(q: np.ndarray, k: np.ndarray, v: np.ndarray) -> np.ndarray:
    from concourse.bass_utils import run_bass_kernel_spmd

    assert q.shape == (B, H, N, DH)
    if "nc" not in _NC_CACHE:
        _NC_CACHE["nc"] = build_nc()
    nc = _NC_CACHE["nc"]

    qt = _pack_qk(np.asarray(q, dtype=np.float32).reshape(B * H, N, DH))
    kt = _pack_qk(np.asarray(k, dtype=np.float32).reshape(B * H, N, DH))
    vt = _pack_v(np.asarray(v, dtype=np.float32).reshape(B * H, N, DH))
    hpc = HEADS
    in_maps = [
        {
            "q": np.ascontiguousarray(qt[c * hpc : (c + 1) * hpc]),
            "k": np.ascontiguousarray(kt[c * hpc : (c + 1) * hpc]),
            "v": np.ascontiguousarray(vt[c * hpc : (c + 1) * hpc]),
        }
        for c in range(NCORES)
    ]
    res = run_bass_kernel_spmd(nc, in_maps, core_ids=list(range(NCORES)))
    out = np.concatenate([res.results[c]["out"] for c in range(NCORES)], axis=0)
    # [64, seq, ball, d] fp16 -> [B, H, N, DH] fp32
    out = out.transpose(0, 2, 1, 3).reshape(B, H, N, DH)
    return out.astype(np.float32)
